# revision 18
# baseline (speedup 1.0000x reference)
"""EntNetHead Trainium2 kernel (v3).

Data-parallel over batch B=64 across 8 NeuronCores (8 batch rows per core);
T=256 recurrent steps run on-chip per core.

Per-core layout:
  - packed elementwise layout [128, 384]: partitions 0..39 hold rows
    (k*8+b) for hidden dims 0:384, partitions 64..103 the same rows for
    hidden dims 384:768.
  - hT [128, 3*128] fp16: transposed UNNORMALIZED hu per 128-chunk (PE
    stationary).  The normalization scalar rn_t = 1/||hu_t|| is factored
    out of the recurrence: z_t = rn_{t-1} * (hu_{t-1} @ U) + c_t, the
    gate gram is rescaled inside the sigmoid (scale=rn, bias=s.keys),
    and scores are rescaled at copy time.  This takes the norm/rsqrt
    chain off the per-step critical path.
  - sqT [128, 6*(2*NR+16)] fp16: per chunk c, step slot t holds 16 cols
    [q_{t-1} (8) | s_t (8)] so gate+score grams share one matmul.

Per step critical chain: z burst (PE) -> zc = rn*z + c (DVE) ->
cand = g*prelu (ACT) -> hu = h + cand (DVE) -> transposes (PE) ->
hT copy (DVE).  Off-chain in parallel: norm^2, P64 cross-half sums,
sqrt, reciprocal, h = rn*hu, sigmoid, scores.  Filler matmuls keep the
PE HAM clock-gate warm.
"""

import sys

sys.path.insert(0, "/opt/trn_rl_repo")

from contextlib import ExitStack

import numpy as np

import concourse.bacc as bacc
import concourse.bass as bass
import concourse.tile as tile
from concourse import mybir
from concourse.bass_utils import run_bass_kernel_spmd

F32 = mybir.dt.float32
F16 = mybir.dt.float16
I32 = mybir.dt.int32
ALU = mybir.AluOpType
ACTF = mybir.ActivationFunctionType
AXX = mybir.AxisListType.X

T, B, H, K, L = 256, 64, 768, 5, 3
NC = 8
BL = B // NC          # 8 batch rows per core
R = K * BL            # 40 (k,b) rows
RP = 64               # padded stationary width
HC = H // 128         # 6 contraction chunks
HH = H // 2           # 384


def _host_consts():
    selK = np.zeros((K, RP), np.float16)
    for k in range(K):
        selK[k, k * BL:(k + 1) * BL] = 1.0
    selB = np.zeros((128, 16 * RP), np.float16)
    for m in range(16):
        for b in range(BL):
            for k in range(K):
                selB[m * BL + b, m * RP + k * BL + b] = 1.0
    maskQG = np.zeros((128, 16), np.float16)
    for p in range(128):
        maskQG[p, p % BL] = 1.0
        maskQG[p, 8 + p % BL] = 1.0
    maskB = np.zeros((128, 512), np.float16)
    for p in range(128):
        for j in range(64):
            maskB[p, j * BL + p % BL] = 1.0
    I64 = np.zeros((128, RP), np.float16)
    for j in range(RP):
        I64[j, j] = 1.0
        I64[64 + j, j] = 1.0
    I128 = np.eye(128, dtype=np.float32)
    selK32 = selK.astype(np.float32)
    mask24 = np.zeros((R, BL * L), np.float32)
    for k in range(K):
        for b in range(BL):
            mask24[k * BL + b, b * L:(b + 1) * L] = 1.0
    ones1x128 = np.ones((1, 128), np.float32)
    P64 = np.zeros((128, 128), np.float32)
    for i in range(128):
        P64[i, i % 64] = 1.0
        P64[i, i % 64 + 64] = 1.0
    return {
        "c_selK": selK, "c_selB32": selB, "c_maskQG": maskQG,
        "c_maskB": maskB, "c_I64": I64, "c_I128": I128, "c_selK32": selK32,
        "c_mask24": mask24, "c_ones": ones1x128, "c_P64": P64,
        "c_I128h": np.eye(128, dtype=np.float16),
    }


def _build(nsteps, n_fill=10, n_fill2=4, rewarm=12, n_rewarm=24):
    nc = bacc.Bacc("TRN2", target_bir_lowering=False, debug=False)
    NR = nsteps * BL      # feature rows per core
    ntt = (NR + 127) // 128
    BLK = 2 * NR + 16     # sqT cols per chunk: [q_{t-1}|s_t] slots + final q

    d_fs = nc.dram_tensor("features_sentence", [nsteps, BL, H], F32, kind="ExternalInput")
    d_fe = nc.dram_tensor("features_entity", [nsteps, BL, H], F32, kind="ExternalInput")
    d_keys = nc.dram_tensor("keys", [K, H], F32, kind="ExternalInput")
    d_U = nc.dram_tensor("U", [H, H], F32, kind="ExternalInput")
    d_V = nc.dram_tensor("V", [H, H], F32, kind="ExternalInput")
    d_W = nc.dram_tensor("W", [H, H], F32, kind="ExternalInput")
    d_alpha = nc.dram_tensor("alpha", [1], F32, kind="ExternalInput")
    d_Wout = nc.dram_tensor("W_out", [K, L], F32, kind="ExternalInput")
    d_bout = nc.dram_tensor("b_out", [L], F32, kind="ExternalInput")
    d_selK = nc.dram_tensor("c_selK", [K, RP], F16, kind="ExternalInput")
    d_selB32 = nc.dram_tensor("c_selB32", [128, 16 * RP], F16, kind="ExternalInput")
    d_maskQG = nc.dram_tensor("c_maskQG", [128, 16], F16, kind="ExternalInput")
    d_maskB = nc.dram_tensor("c_maskB", [128, 512], F16, kind="ExternalInput")
    d_I64 = nc.dram_tensor("c_I64", [128, RP], F16, kind="ExternalInput")
    d_I128 = nc.dram_tensor("c_I128", [128, 128], F32, kind="ExternalInput")
    d_selK32 = nc.dram_tensor("c_selK32", [K, RP], F32, kind="ExternalInput")
    d_mask24 = nc.dram_tensor("c_mask24", [R, BL * L], F32, kind="ExternalInput")
    d_ones = nc.dram_tensor("c_ones", [1, 128], F32, kind="ExternalInput")
    d_P64 = nc.dram_tensor("c_P64", [128, 128], F32, kind="ExternalInput")
    d_I128h = nc.dram_tensor("c_I128h", [128, 128], F16, kind="ExternalInput")
    d_out = nc.dram_tensor("preds", [NR, L], F32, kind="ExternalOutput")

    with tile.TileContext(nc) as tc, ExitStack() as ctx:
        ep = ctx.enter_context

        p_sqT = ep(tc.tile_pool(name="sqT", bufs=1))
        p_sW = ep(tc.tile_pool(name="sW", bufs=1))
        p_prm = ep(tc.tile_pool(name="prm", bufs=1))
        p_h = ep(tc.tile_pool(name="h", bufs=2))
        p_hT = ep(tc.tile_pool(name="hT", bufs=2))
        p_e16 = ep(tc.tile_pool(name="e16", bufs=2))
        p_sml = ep(tc.tile_pool(name="sml", bufs=3))

        sqT = p_sqT.tile([128, HC * BLK], F16)
        sW = p_sW.tile([128, ntt * H], F16)   # [row%128, tile*768+h]
        U16 = p_prm.tile([128, HC * H], F16, tag="U16")
        keyV = p_prm.tile([128, H], F16, tag="keyV")
        keys16 = p_prm.tile([128, H], F16, tag="keys16")
        keysT = p_prm.tile([128, HC * 8], F16, tag="keysT")
        sKb = p_prm.tile([128, nsteps], F32, tag="sKb")
        selK16 = p_prm.tile([128, RP], F16, tag="selK16")
        selB32 = p_prm.tile([128, 16 * RP], F16, tag="selB32")
        maskQG = p_prm.tile([128, 16], F16, tag="maskQG")
        maskB = p_prm.tile([128, 512], F16, tag="maskB")
        I64 = p_prm.tile([128, RP], F16, tag="I64")
        I128 = p_prm.tile([128, 128], F32, tag="I128")
        selK32 = p_prm.tile([128, RP], F32, tag="selK32")
        mask24 = p_prm.tile([128, BL * L], F32, tag="mask24")
        ones_r = p_prm.tile([128, 128], F32, tag="ones_r")
        P64 = p_prm.tile([128, 128], F32, tag="P64")
        I128h = p_prm.tile([128, 128], F16, tag="I128h")
        alphav = p_prm.tile([128, 1], F32, tag="alphav")
        epsv = p_prm.tile([128, 1], F32, tag="epsv")
        onev = p_prm.tile([128, 1], F32, tag="onev")
        Wsel = p_prm.tile([128, BL * L], F32, tag="Wsel")
        bvec = p_prm.tile([128, 1], F32, tag="bvec")
        scores = p_prm.tile([128, nsteps], F32, tag="scores")

        dma = nc.sync.dma_start

        dma(selK16[0:K, :], d_selK.ap())
        dma(selB32[:, :], d_selB32.ap())
        dma(maskQG[:, :], d_maskQG.ap())
        dma(maskB[:, :], d_maskB.ap())
        dma(I64[:, :], d_I64.ap())
        dma(I128[:, :], d_I128.ap())
        dma(selK32[0:K, :], d_selK32.ap())
        dma(mask24[0:R, :], d_mask24.ap())
        dma(ones_r[0:1, :], d_ones.ap())
        dma(P64[:, :], d_P64.ap())
        dma(I128h[:, :], d_I128h.ap())
        for b in range(BL):
            dma(bvec[b * L:(b + 1) * L, 0:1], bass.AP(d_bout, 0, [[1, L], [1, 1]]))
        nc.vector.memset(epsv[:, :], 1e-12)
        nc.vector.memset(onev[:, :], 1.0)

        def hts(hT_tile, c):
            if c < 3:
                return hT_tile[:, c * 128:c * 128 + RP]
            return hT_tile[:, (c - 3) * 128 + RP:(c - 2) * 128]

        def emit_transposes(h_tile, out_psum):
            for cc in range(3):
                nc.tensor.transpose(out_psum[:, cc * 128:(cc + 1) * 128],
                                    h_tile[:, cc * 128:(cc + 1) * 128],
                                    I128h[:, :])

        with tc.tile_pool(name="pr32", bufs=1) as p32, \
             tc.tile_pool(name="prps", bufs=4, space="PSUM") as pps:

            def ptile(shape, dt):
                return pps.tile(shape, dt, tag="ps", name="ps")

            sT = p32.tile([128, HC * NR], F16, tag="sT", name="sT", bufs=1)

            # keys
            kn = p32.tile([128, H], F32, tag="kn")
            dma(kn[0:K, :], d_keys.ap())
            nc.vector.tensor_copy(keys16[0:K, :], kn[0:K, :])
            # U
            un = p32.tile([128, HC * H], F32, tag="big")
            for c in range(HC):
                dma(un[:, c * H:(c + 1) * H], d_U.ap()[c * 128:(c + 1) * 128, :])
            nc.vector.tensor_copy(U16[:, :], un[:, :])
            # keysT via PE transpose
            tps = ptile([128, HC * 8], F16)
            for c in range(HC):
                nc.tensor.transpose(tps[:, c * 8:c * 8 + K],
                                    keys16[0:K, c * 128:(c + 1) * 128],
                                    I64[0:K, 0:K])
            for c in range(HC):
                nc.vector.tensor_copy(keysT[:, c * 8:c * 8 + K],
                                      tps[:, c * 8:c * 8 + K])
            # V -> keyV = keys @ V
            vn = p32.tile([128, HC * H], F32, tag="big")
            for c in range(HC):
                dma(vn[:, c * H:(c + 1) * H], d_V.ap()[c * 128:(c + 1) * 128, :])
            v16 = p32.tile([128, HC * H], F16, tag="big16")
            nc.vector.tensor_copy(v16[:, :], vn[:, :])
            for i in range(2):
                kvp = ptile([128, HH], F32)
                for c in range(HC):
                    nc.tensor.matmul(kvp[0:K, :],
                                     keysT[:, c * 8:c * 8 + K],
                                     v16[:, c * H + i * HH: c * H + (i + 1) * HH],
                                     start=(c == 0), stop=(c == HC - 1))
                nc.vector.tensor_copy(keyV[0:K, i * HH:(i + 1) * HH], kvp[0:K, :])
            # W (fp16) for sW matmuls
            wn = p32.tile([128, HC * H], F32, tag="big")
            for c in range(HC):
                dma(wn[:, c * H:(c + 1) * H], d_W.ap()[c * 128:(c + 1) * 128, :])
            w16 = p32.tile([128, HC * H], F16, tag="big16w")
            nc.vector.tensor_copy(w16[:, :], wn[:, :])

            # alpha -> all partitions
            asb = p32.tile([128, 1], F32, tag="asb")
            dma(asb[0:1, 0:1], bass.AP(d_alpha, 0, [[1, 1], [1, 1]]))
            alp = ptile([128, 1], F32)
            nc.tensor.matmul(alp[:, :], ones_r[0:1, :], asb[0:1, 0:1],
                             start=True, stop=True)
            nc.vector.tensor_copy(alphav[:, :], alp[:, :])

            # W_out -> Wsel
            wo = p32.tile([128, L], F32, tag="wo")
            dma(wo[0:K, :], d_Wout.ap())
            wrp = ptile([128, L], F32)
            nc.tensor.matmul(wrp[0:R, :], selK32[0:K, 0:R], wo[0:K, :],
                             start=True, stop=True)
            wrs = p32.tile([128, L], F32, tag="wrs")
            nc.vector.tensor_copy(wrs[0:R, :], wrp[0:R, :])
            for b in range(BL):
                nc.vector.tensor_mul(Wsel[0:R, b * L:(b + 1) * L],
                                     wrs[0:R, :], mask24[0:R, b * L:(b + 1) * L])

            # zero q-slot 0 and the s-part of the final slot of sqT
            for c in range(HC):
                nc.vector.memset(sqT[:, c * BLK:c * BLK + 8], 0.0)
                nc.vector.memset(sqT[:, c * BLK + 2 * NR + 8:(c + 1) * BLK], 0.0)

            # features: DMA, transpose into sqT slots + contiguous sT,
            # and sW for the sentence stream
            def do_feat(dram, is_q):
                for tb in range(ntt):
                    fn = p32.tile([128, H], F32, tag="fnat")
                    nrow = min(128, NR - tb * 128)
                    src = bass.AP(dram, tb * 128 * H, [[H, nrow], [1, H]])
                    dma(fn[0:nrow, :], src)
                    for grp in range(2):
                        tp = ptile([128, 3 * 128], F32)
                        for j in range(3):
                            c = grp * 3 + j
                            nc.tensor.transpose(
                                tp[:, j * nrow:(j + 1) * nrow],
                                fn[0:nrow, c * 128:(c + 1) * 128],
                                I128[0:nrow, 0:nrow])
                        # slot col: q_t -> slot t+1 cols 0:8; s_t -> slot t cols 8:16
                        off = (grp * 3) * BLK + tb * 16 * 16 + (16 if is_q else 8)
                        dst = bass.AP(
                            sqT.tensor,
                            sqT.offset + off,
                            [[HC * BLK, 128], [BLK, 3], [16, 16], [1, BL]])
                        nc.vector.tensor_copy(
                            dst,
                            tp[:, 0:3 * nrow].rearrange(
                                "p (a b c) -> p a b c", a=3, b=16))
                        if not is_q:
                            dstc = bass.AP(
                                sT.tensor,
                                sT.offset + (grp * 3) * NR + tb * 128,
                                [[HC * NR, 128], [NR, 3], [1, nrow]])
                            nc.vector.tensor_copy(
                                dstc,
                                tp[:, 0:3 * nrow].rearrange(
                                    "p (a b) -> p a b", a=3))
                    if not is_q:
                        for i in range(2):
                            swp = ptile([128, HH], F32)
                            for c in range(HC):
                                lhs = sT[:, c * NR + tb * 128:
                                         c * NR + tb * 128 + nrow]
                                nc.tensor.matmul(
                                    swp[0:nrow, :], lhs,
                                    w16[:, c * H + i * HH:c * H + (i + 1) * HH],
                                    start=(c == 0), stop=(c == HC - 1))
                            nc.vector.tensor_copy(
                                sW[0:nrow, tb * H + i * HH:tb * H + (i + 1) * HH],
                                swp[0:nrow, :])

            do_feat(d_fs, False)
            do_feat(d_fe, True)

            # h0 = keys broadcast to (k,b) rows
            h0p = ptile([128, HH], F32)
            nc.tensor.matmul(h0p[0:RP, :], selK16[0:K, :], keys16[0:K, 0:HH],
                             start=True, stop=True, skip_group_check=True)
            nc.tensor.matmul(h0p[64:128, :], selK16[0:K, :], keys16[0:K, HH:H],
                             start=True, stop=True, skip_group_check=True)
            h_cur = p_h.tile([128, HH], F16, tag="h")
            nc.vector.tensor_copy(h_cur[:, :], h0p[:, :])

            tp0 = ptile([128, 3 * 128], F16)
            emit_transposes(h_cur, tp0)
            hT_cur = p_hT.tile([128, 3 * 128], F16, tag="hT")
            nc.vector.tensor_copy(hT_cur[:, :], tp0[:, :])

            # sKb[p=(k,b), t] = keys[k] . s_t[b]: gram of sT against the
            # transposed keys broadcast (hT_cur == transpose(h0) here)
            for q in range((NR + 511) // 512):
                ncol = min(512, NR - q * 512)
                nst = ncol // BL
                skp = ptile([128, 512], F32)
                for c in range(HC):
                    half = skp[0:RP, 0:ncol] if c < 3 else skp[64:128, 0:ncol]
                    nc.tensor.matmul(half, hts(hT_cur, c),
                                     sT[:, c * NR + q * 512:c * NR + q * 512 + ncol],
                                     start=(c % 3 == 0), stop=(c % 3 == 2),
                                     skip_group_check=True)
                skm = p32.tile([128, 512], F16, tag="skm")
                skr = p32.tile([128, 64], F32, tag="skr")
                nc.vector.tensor_mul(skm[:, 0:ncol], skp[:, 0:ncol],
                                     maskB[:, 0:ncol])
                nc.vector.tensor_reduce(
                    skr[:, 0:nst],
                    skm[:, 0:ncol].rearrange("p (a b) -> p a b", a=nst),
                    AXX, ALU.add)
                skp2 = ptile([128, 64], F32)
                nc.tensor.matmul(skp2[:, 0:nst], P64[:, :], skr[:, 0:nst],
                                 start=True, stop=True)
                nc.vector.tensor_copy(sKb[:, q * 64:q * 64 + nst],
                                      skp2[:, 0:nst])

        # ---- main loop ----
        p_zps = ep(tc.tile_pool(name="zps", bufs=2, space="PSUM"))
        p_cps = ep(tc.tile_pool(name="cps", bufs=2, space="PSUM"))
        p_gps = ep(tc.tile_pool(name="gps", bufs=2, space="PSUM"))
        p_tps = ep(tc.tile_pool(name="tps", bufs=1, space="PSUM"))
        p_fps = ep(tc.tile_pool(name="fps", bufs=1, space="PSUM"))

        fillP = p_fps.tile([128, HH], F32, tag="fill")

        def inject(cP, t):
            tb, m = (t * BL) // 128, t % 16
            nc.tensor.matmul(cP[0:RP, :], selK16[0:K, :], keyV[0:K, 0:HH],
                             start=True, stop=False, skip_group_check=True)
            nc.tensor.matmul(cP[64:128, :], selK16[0:K, :], keyV[0:K, HH:H],
                             start=True, stop=False, skip_group_check=True)
            lsel = selB32[:, m * RP:(m + 1) * RP]
            nc.tensor.matmul(cP[0:RP, :], lsel,
                             sW[:, tb * H:tb * H + HH],
                             start=False, stop=True, skip_group_check=True)
            nc.tensor.matmul(cP[64:128, :], lsel,
                             sW[:, tb * H + HH:(tb + 1) * H],
                             start=False, stop=True, skip_group_check=True)

        def filler(n):
            for _ in range(n):
                nc.tensor.matmul(fillP[0:RP, :], selB32[:, 0:RP], U16[:, 0:HH],
                                 start=True, stop=True, skip_group_check=True)

        filler(30)          # warm-up: trip the HAM SHORT window before step 0
        cP_next = p_cps.tile([128, HH], F32, tag="c")
        inject(cP_next, 0)
        rn_prev = onev

        for t in range(nsteps):
            cP = cP_next
            zP = p_zps.tile([128, HH], F32, tag="z")
            gqP = p_gps.tile([128, 16], F32, tag="gq", name="gqP")

            # grams: [q_{t-1} | s_t] vs hu_{t-1}, halves split over chunks
            for c in range(3):
                mvA = sqT[:, c * BLK + t * 16:c * BLK + t * 16 + 16]
                mvB = sqT[:, (c + 3) * BLK + t * 16:(c + 3) * BLK + t * 16 + 16]
                nc.tensor.matmul(gqP[0:RP, 0:16], hts(hT_cur, c), mvA,
                                 start=(c == 0), stop=(c == 2),
                                 skip_group_check=True)
                nc.tensor.matmul(gqP[64:128, 0:16], hts(hT_cur, c + 3), mvB,
                                 start=(c == 0), stop=(c == 2),
                                 skip_group_check=True)

            # z = hu_{t-1} @ U, alternating halves
            for c in range(HC):
                lhs = hts(hT_cur, c)
                nc.tensor.matmul(zP[0:RP, :], lhs, U16[:, c * H:c * H + HH],
                                 start=(c == 0), stop=(c == HC - 1),
                                 skip_group_check=True)
                nc.tensor.matmul(zP[64:128, :], lhs,
                                 U16[:, c * H + HH:(c + 1) * H],
                                 start=(c == 0), stop=(c == HC - 1),
                                 skip_group_check=True)

            # DVE: masked reduce of [q|g] -> gq2 [128,2]
            gq16 = p_sml.tile([128, 16], F16, tag="gq16")
            gq2 = p_sml.tile([128, 2], F32, tag="gq2")
            nc.vector.tensor_mul(gq16[:, :], gqP[:, :], maskQG[:, :])
            nc.vector.tensor_reduce(gq2[:, :],
                                    gq16[:, :].rearrange("p (a b) -> p a b", a=2),
                                    AXX, ALU.add)
            # PE: cross-half sum [q|g]
            paP = p_gps.tile([128, 2], F32, tag="gq", name="paP")
            nc.tensor.matmul(paP[:, :], P64[:, :], gq2[:, :], start=True, stop=True)

            # next-step c injections + fillers keep PE busy during the chain
            if t + 1 < nsteps:
                cP_next = p_cps.tile([128, HH], F32, tag="c")
                inject(cP_next, t + 1)
            filler(n_fill)

            # gate = sigmoid(rn_{t-1} * (s.hu) + s.keys)
            gsig = p_sml.tile([128, 1], F32, tag="gsig")
            nc.scalar.activation(gsig[:, :], paP[:, 1:2], ACTF.Sigmoid,
                                 scale=rn_prev[:, :], bias=sKb[:, t:t + 1])
            if t > 0:
                nc.vector.tensor_scalar(scores[:, t - 1:t], paP[:, 0:1],
                                        rn_prev[:, :], None, ALU.mult)
            # zc = rn_{t-1} * zP + cP ; cand = gsig * prelu(zc), split in
            # column halves [0:256|256:384] so DVE/ACT pipeline and the
            # first two transposes start before the second half finishes
            zc = p_e16.tile([128, HH], F16, tag="zc")
            zcc = p_e16.tile([128, HH], F16, tag="zcc")
            cand = p_e16.tile([128, HH], F16, tag="cand")
            hu = p_e16.tile([128, HH], F16, tag="hu")
            tP = p_tps.tile([128, 3 * 128], F16, tag="t")
            hT_new = p_hT.tile([128, 3 * 128], F16, tag="hT")
            splits = [(0, 256), (256, HH)]
            for lo, hi in splits:
                nc.vector.tensor_scalar(zc[:, lo:hi], zP[:, lo:hi],
                                        rn_prev[:, :], None, ALU.mult)
                nc.vector.tensor_add(zcc[:, lo:hi], zc[:, lo:hi], cP[:, lo:hi])
                nc.scalar.activation(cand[:, lo:hi], zcc[:, lo:hi], ACTF.Prelu,
                                     scale=gsig[:, :], alpha=alphav[:, :])
                nc.vector.tensor_add(hu[:, lo:hi], h_cur[:, lo:hi],
                                     cand[:, lo:hi])
                for cc in range(lo // 128, hi // 128):
                    nc.tensor.transpose(tP[:, cc * 128:(cc + 1) * 128],
                                        hu[:, cc * 128:(cc + 1) * 128],
                                        I128h[:, :])
            nc.vector.tensor_copy(hT_new[:, :], tP[:, :])

            # off-chain: norm^2 (ACT square+accum), cross-half sum, rsqrt,
            # h = rn*hu
            squ = p_e16.tile([128, HH], F16, tag="squ")
            ss = p_sml.tile([128, 1], F32, tag="ss")
            nc.scalar.activation(squ[:, :], hu[:, :], ACTF.Square,
                                 accum_out=ss[:, :])
            paB = p_gps.tile([128, 1], F32, tag="gq", name="paB")
            nc.tensor.matmul(paB[:, :], P64[:, :], ss[:, :], start=True, stop=True)
            filler(n_fill2)
            if rewarm and t % rewarm == rewarm - 1:
                filler(n_rewarm)
            # rn = rsqrt(ss): bit-trick + 1 Newton step, all DVE, off-chain
            # (an ACT Sqrt would evict the Sigmoid/Prelu/Square table group
            # and put a 1.3us table reload on the critical path)
            sdi = p_sml.tile([128, 1], I32, tag="sdi")
            nc.vector.tensor_scalar(sdi[:, :], paB.bitcast(I32)[:, :], 1, None,
                                    ALU.logical_shift_right)
            nc.vector.tensor_scalar(sdi[:, :], sdi[:, :], -1, 0x5F3759DF,
                                    ALU.mult, ALU.add)
            rn = sdi.bitcast(F32)
            ra = p_sml.tile([128, 1], F32, tag="ra")
            nc.vector.tensor_mul(ra[:, :], rn[:, :], paB[:, :])
            nc.vector.tensor_mul(ra[:, :], ra[:, :], rn[:, :])
            nc.vector.tensor_scalar(ra[:, :], ra[:, :], -0.5, 1.5,
                                    ALU.mult, ALU.add)
            nc.vector.tensor_mul(rn[:, :], rn[:, :], ra[:, :])
            h_new = p_h.tile([128, HH], F16, tag="h")
            nc.vector.tensor_scalar(h_new[:, :], hu[:, :], rn[:, :], None,
                                    ALU.mult)
            h_cur, hT_cur, rn_prev = h_new, hT_new, rn

        # epilogue: last score rn_{T-1}*(q_{T-1} . hu_{T-1}) + output head
        gqF = p_gps.tile([128, 16], F32, tag="gq", name="gqF")
        for c in range(3):
            mvA = sqT[:, c * BLK + nsteps * 16:c * BLK + nsteps * 16 + 8]
            mvB = sqT[:, (c + 3) * BLK + nsteps * 16:(c + 3) * BLK + nsteps * 16 + 8]
            nc.tensor.matmul(gqF[0:RP, 0:8], hts(hT_cur, c), mvA,
                             start=(c == 0), stop=(c == 2), skip_group_check=True)
            nc.tensor.matmul(gqF[64:128, 0:8], hts(hT_cur, c + 3), mvB,
                             start=(c == 0), stop=(c == 2), skip_group_check=True)
        gqf16 = p_sml.tile([128, 8], F16, tag="gqf")
        gqf2 = p_sml.tile([128, 1], F32, tag="gqf2")
        nc.vector.tensor_mul(gqf16[:, :], gqF[:, 0:8], maskQG[:, 0:8])
        nc.vector.tensor_reduce(gqf2[:, :], gqf16[:, :], AXX, ALU.add)
        paF = p_gps.tile([128, 2], F32, tag="gq", name="paF")
        nc.tensor.matmul(paF[:, 0:1], P64[:, :], gqf2[:, :], start=True, stop=True)
        nc.vector.tensor_scalar(scores[:, nsteps - 1:nsteps], paF[:, 0:1],
                                rn_prev[:, :], None, ALU.mult)

        pP = p_fps.tile([128, nsteps], F32, tag="fill", name="head")
        nc.tensor.matmul(pP[0:BL * L, :], Wsel[0:R, 0:BL * L],
                         scores[0:R, 0:nsteps], start=True, stop=True)
        osb = p_prm.tile([128, nsteps], F32, tag="osb")
        nc.vector.tensor_scalar(osb[0:BL * L, :], pP[0:BL * L, :],
                                bvec[0:BL * L, :], None, ALU.add)
        nc.sync.dma_start(bass.AP(d_out, 0, [[1, BL * L], [BL * L, nsteps]]),
                          osb[0:BL * L, :])

    nc.compile()
    return nc


_CACHE = {}


def _get(nsteps):
    if nsteps not in _CACHE:
        _CACHE[nsteps] = _build(nsteps)
    return _CACHE[nsteps]


def run(inputs, **spmd_kwargs):
    nsteps = inputs["features_sentence"].shape[0]
    nc = _get(nsteps)
    consts = _host_consts()
    fs = np.ascontiguousarray(np.asarray(inputs["features_sentence"], dtype=np.float32))
    fe = np.ascontiguousarray(np.asarray(inputs["features_entity"], dtype=np.float32))
    shared = {k: np.ascontiguousarray(np.asarray(inputs[k], dtype=np.float32))
              for k in ("keys", "U", "V", "W", "alpha", "W_out", "b_out")}
    shared.update(consts)
    in_maps = []
    for c in range(NC):
        m = dict(shared)
        m["features_sentence"] = np.ascontiguousarray(fs[:, c * BL:(c + 1) * BL, :])
        m["features_entity"] = np.ascontiguousarray(fe[:, c * BL:(c + 1) * BL, :])
        in_maps.append(m)
    res = run_bass_kernel_spmd(nc, in_maps, core_ids=list(range(NC)), **spmd_kwargs)
    outs = [r["preds"].reshape(nsteps, BL, L) for r in res.results]
    return np.concatenate(outs, axis=1).reshape(nsteps * B, L), res


def kernel(**inputs):
    out, _ = run(inputs)
    return out
